# revision 26
# baseline (speedup 1.0000x reference)
"""Block-local self-attention (BLOCK=128, 3-block windows + global token) on 8
Trainium2 NeuronCores.

Sharding: batch*heads = 32 (n,h) pairs -> 4 pairs per core, no cross-core comms.

Device computes ONLY the block-local window attention, unnormalized:
  - QK: per k-block j (32 slabs), one matmul scoresT[k in j, q in 3 blocks]
    (N=384) from a COMPACT Q^T tile (no host-side 3x replication), with the
    additive mask folded in as a 65th contraction row (K-side row = mask,
    Q-side row = 1.0) and 1/sqrt(d) folded into Q on the host.  The
    contraction dim is zero-padded 65->128 so the full-column weights take
    the PE fast-weight-load path.  Token 0's key is masked out (NEG) so the
    window path excludes it.
  - exp on ScalarE -- the bottleneck engine -- batched 3 slabs/op
    (PSUM->SBUF bf16); QK runs two batches ahead and is hoisted across the
    pair boundary so the exp stream never stalls.
  - PV: ctx[q, 0:64] = sum e*V and ctx[q, 64] = sum e (ones column of V')
    accumulated in PSUM over the 2-3 contributing slabs, 4 windows per PSUM
    group tile; even/odd windows go to different groups so interleaved
    accumulation chains never share a PSUM bank.
  - each finished group is copied PSUM->SBUF by the (otherwise idle) DVE
    into a per-pair out tile and shipped immediately (1KB-row DMAs).

Everything global/tiny runs on the host in numpy instead of burning PE
weight-loads on rank-1 matmuls: the token-0 global-slot term
(out = (ctx + e0*V0) / (den + e0)), the softmax normalization, and the
global-query row (token 0 attends to all keys).
"""

import numpy as np
import ml_dtypes

N, H, T, D = 2, 16, 4000, 64
BLOCK = 128
TP = 4096            # padded token count (32 blocks)
W = 32               # number of 128-blocks
NCORES = 8
PAIRS = N * H        # 32
PPC = PAIRS // NCORES  # pairs per core
NEG = -30000.0
SCALE = 1.0 / np.sqrt(np.float32(D))

# window w -> (group, slot): group = (w%2)*4 + w//8, slot = (w//2)%4
_GRP = [(w % 2) * 4 + w // 8 for w in range(W)]
_SLOT = [(w // 2) % 4 for w in range(W)]
# group -> column position in the out tile, ordered by completion time
# (g0 done at m=3, g4 at m=4, g1 at m=7, ...) so finished halves of the out
# tile can be DMA'd in 2-group chunks while the pair is still computing.
_GORDER = [0, 4, 1, 5, 2, 6, 3, 7]
_GOFF = [_GORDER.index(g) * 260 for g in range(8)]

_prog_cache = {}


def _qlo(j):
    return min(max(j - 1, 0), W - 3)


def _slabs(w):
    return [s for s in (w - 1, w, w + 1) if 0 <= s < W]


def _build_program():
    if "nc" in _prog_cache:
        return _prog_cache["nc"]

    import concourse.bacc as bacc
    import concourse.mybir as mybir
    from concourse import tile

    dt = mybir.dt
    EXP = mybir.ActivationFunctionType.Exp

    nc = bacc.Bacc("TRN2", target_bir_lowering=False, debug=False,
                   num_devices=NCORES)
    # contraction dim padded 65 -> 128 (zero rows): full-column weights
    # enable the PE fast-weight-load path, hiding QK LDWEIGHTS.
    qte_d = nc.dram_tensor("qte", [PPC, 128, TP], dt.bfloat16,
                           kind="ExternalInput").ap()
    kte_d = nc.dram_tensor("kte", [PPC, 128, TP], dt.bfloat16,
                           kind="ExternalInput").ap()
    vp_d = nc.dram_tensor("vp", [PPC, 128, W * 65], dt.bfloat16,
                          kind="ExternalInput").ap()
    out_d = nc.dram_tensor("out", [PPC, 128, 8 * 260], dt.float32,
                           kind="ExternalOutput").ap()

    with tile.TileContext(nc) as tc:
        with (
            tc.tile_pool(name="qte", bufs=3) as qte_pool,
            tc.tile_pool(name="kte", bufs=3) as kte_pool,
            tc.tile_pool(name="vp", bufs=3) as vp_pool,
            tc.tile_pool(name="ex", bufs=4) as ex_pool,
            tc.tile_pool(name="small", bufs=2) as small_pool,
            tc.tile_pool(name="outp", bufs=2) as out_pool,
            tc.tile_pool(name="sc", bufs=2, space="PSUM") as sc_pool,
            tc.tile_pool(name="ctx", bufs=2, space="PSUM") as ctx_pool,
        ):
            def load_pair(p, split=False):
                kte_t = kte_pool.tile([128, TP], dt.bfloat16, tag="kte",
                                      name=f"kte_{p}")
                qte_t = qte_pool.tile([128, TP], dt.bfloat16, tag="qte",
                                      name=f"qte_{p}")
                vp_t = vp_pool.tile([128, W * 65], dt.bfloat16, tag="vp",
                                    name=f"vp_{p}")
                if split:
                    # pair 0 gates the whole pipeline: land the first-QK
                    # columns first so compute starts earlier, and keep the
                    # chunks fine so the QK stream never outruns the DMA
                    # (a >1.7us PE stall here re-throttles the PE clock)
                    for c in range(4):
                        lo, hi = c * 1024, (c + 1) * 1024
                        nc.sync.dma_start(kte_t[:, lo:hi], kte_d[p][:, lo:hi])
                        nc.sync.dma_start(qte_t[:, lo:hi], qte_d[p][:, lo:hi])
                else:
                    nc.sync.dma_start(kte_t[:], kte_d[p])
                    nc.sync.dma_start(qte_t[:], qte_d[p])
                nc.sync.dma_start(vp_t[:], vp_d[p])
                return qte_t, kte_t, vp_t

            # PE warm-up: dense N=512 matmuls on memset data trip the HAM
            # un-throttle (~3.4us of sustained activity) while the first
            # pair's inputs stream in.
            warm_sb = small_pool.tile([128, 512], dt.bfloat16, tag="warm")
            nc.gpsimd.memset(warm_sb[:], 0.25)
            warm_ps = sc_pool.tile([128, 1536], dt.float32, tag="sc",
                                   name="warm_ps")
            for r in range(4):
                nc.tensor.matmul(warm_ps[:, 0:512], warm_sb[:, 0:128],
                                 warm_sb[:, 0:512], start=True, stop=True)

            pending = {0: load_pair(0, split=True)}
            hoisted = {}
            for p in range(PPC):
                qte_t, kte_t, vp_t = pending.pop(p)
                if p + 1 < PPC:
                    pending[p + 1] = load_pair(p + 1)

                out_t = out_pool.tile([128, 8 * 260], dt.float32, tag="out",
                                      name=f"out_{p}")
                ex_tiles = {}
                ctx_tiles = {}

                def emit_qk(b, qte_t=qte_t, kte_t=kte_t, p=p):
                    # scores for slab batch b: k-blocks 3b..min(3b+2, 31)
                    sc = sc_pool.tile([128, 1536], dt.float32, tag="sc",
                                      name=f"sc_{p}_{b}")
                    for h in range(3 if 3 * b + 2 < W else W - 3 * b):
                        j = 3 * b + h
                        lo = _qlo(j) * 128
                        nc.tensor.matmul(
                            sc[:, h * 512:h * 512 + 384],
                            kte_t[:, j * 128:(j + 1) * 128],
                            qte_t[:, lo:lo + 384],
                            start=True, stop=True)
                    return sc

                def emit_exp(b, sc, ex_tiles=ex_tiles, p=p):
                    nb = 3 if 3 * b + 2 < W else W - 3 * b
                    ex = ex_pool.tile([128, 3 * 384], dt.bfloat16, tag="ex",
                                      name=f"ex_{p}_{b}")
                    nc.scalar.activation(
                        ex[:, 0:nb * 384].rearrange("p (b x) -> p b x", x=384),
                        sc[:, 0:nb * 512].rearrange(
                            "p (b x) -> p b x", x=512)[:, :, 0:384],
                        EXP)
                    ex_tiles[b] = ex

                def consume(b, p=p):
                    # windows whose last slab (w+1) landed in batch b
                    ws = [w for w in (3 * b - 1, 3 * b, 3 * b + 1)
                          if 0 <= w < W]
                    # the 1st and 3rd window may share a PSUM group tile:
                    # their accumulation chains must not interleave (two
                    # concurrently open accumulation groups on one bank
                    # corrupt PSUM), so emit the 3rd in its own phase
                    emit_chains(ws[:2])
                    if len(ws) > 2:
                        emit_chains(ws[2:])

                def emit_chains(ws, p=p, vp_t=vp_t, ex_tiles=ex_tiles,
                                ctx_tiles=ctx_tiles, out_t=out_t):
                    seqs = {}
                    for w in ws:
                        g = _GRP[w]
                        if _SLOT[w] == 0:
                            ctx_tiles[g] = ctx_pool.tile(
                                [128, 4 * 65], dt.float32, tag="ctx",
                                name=f"ctx_{p}_{g}")
                        slabs = _slabs(w)
                        seq = []
                        for idx, s in enumerate(slabs):
                            gcol = w - _qlo(s)
                            exm = ex_tiles[s // 3]
                            base = (s % 3) * 384 + gcol * 128
                            seq.append((exm[:, base:base + 128],
                                        vp_t[:, s * 65:(s + 1) * 65],
                                        idx == 0, idx == len(slabs) - 1))
                        seqs[w] = seq
                    # interleave the windows' accumulation chains so
                    # consecutive PE matmuls hit different PSUM banks
                    for r in range(max(len(s) for s in seqs.values())):
                        for w in ws:
                            if r < len(seqs[w]):
                                lhsT, rhs, st, sp = seqs[w][r]
                                g, sl = _GRP[w], _SLOT[w]
                                nc.tensor.matmul(
                                    ctx_tiles[g][:, sl * 65:(sl + 1) * 65],
                                    lhsT, rhs, start=st, stop=sp)
                    for w in ws:
                        g = _GRP[w]
                        if _SLOT[w] == 3:
                            off = _GOFF[g]
                            nc.vector.tensor_copy(
                                out_t[:, off:off + 260], ctx_tiles[g][:])
                            # ship each finished group while the pair is
                            # still computing; rows are 1040B descriptors
                            nc.sync.dma_start(
                                out_d[p][:, off:off + 260],
                                out_t[:, off:off + 260])

                # software pipeline: QK two batches (6 slabs) ahead so the
                # (bottleneck) exp engine never starves on scores.
                NB = (W + 2) // 3
                scs = hoisted.pop(p, None)
                if scs is None:
                    scs = {0: emit_qk(0), 1: emit_qk(1)}
                for b in range(NB):
                    emit_exp(b, scs.pop(b))
                    if b + 2 < NB:
                        scs[b + 2] = emit_qk(b + 2)
                    if 1 <= b <= NB - 2:
                        consume(b - 1)
                # tail: hoist the next pair's first QK batches between the
                # last consumes so the exp engine has no pair-boundary gap
                if p + 1 < PPC:
                    nq, nk, _ = pending[p + 1]
                    h = {0: emit_qk(0, qte_t=nq, kte_t=nk, p=p + 1)}
                    consume(NB - 2)
                    h[1] = emit_qk(1, qte_t=nq, kte_t=nk, p=p + 1)
                    consume(NB - 1)
                    hoisted[p + 1] = h
                else:
                    consume(NB - 2)
                    consume(NB - 1)

    nc.compile()
    _prog_cache["nc"] = nc
    return nc


def _prep_core_inputs(q, k, v, mask):
    """q,k,v: (PAIRS, T, D) f32; mask: (N, T) f32.  Returns list of per-core
    input dicts (bf16 device layouts)."""
    bf16 = ml_dtypes.bfloat16
    maskp = np.repeat(mask, H, axis=0)                   # (PAIRS, T)

    qte = np.zeros((PAIRS, 128, TP), np.float32)
    qte[:, :D, :T] = q.transpose(0, 2, 1) * SCALE
    qte[:, D, :] = 1.0

    kte = np.zeros((PAIRS, 128, TP), np.float32)
    kte[:, :D, :T] = k.transpose(0, 2, 1)
    kte[:, D, :T] = maskp
    kte[:, D, 0] = NEG          # token 0 served by the host global-slot path
    kte[:, D, T:] = NEG

    vp3 = np.zeros((PAIRS, TP, 65), np.float32)
    vp3[:, :T, :D] = v
    vp3[:, :, D] = 1.0
    vp = vp3.reshape(PAIRS, W, 128, 65).transpose(0, 2, 1, 3) \
        .reshape(PAIRS, 128, W * 65)

    qte = qte.astype(bf16)
    kte = kte.astype(bf16)
    vp = vp.astype(bf16)
    return [{
        "qte": qte[c * PPC:(c + 1) * PPC],
        "kte": kte[c * PPC:(c + 1) * PPC],
        "vp": vp[c * PPC:(c + 1) * PPC],
    } for c in range(NCORES)]


def _postprocess(results, q, k, v, mask):
    """Merge the host-side global paths and normalize."""
    maskp = np.repeat(mask, H, axis=0)                   # (PAIRS, T)

    # device windows: (PAIRS, TP, 65) = [sum e*V | sum e]
    o = np.concatenate([results[c]["out"] for c in range(NCORES)], axis=0)
    o = o.reshape(PAIRS, 128, 8, 4, 65)
    pos = [_GOFF[g] // 260 for g in _GRP]
    full = o[:, :, pos, _SLOT, :]                        # (PAIRS, 128, W, 65)
    full = full.transpose(0, 2, 1, 3).reshape(PAIRS, TP, 65)[:, :T]

    # token-0 global slot: every query also attends to k0/v0
    e0 = np.exp((q @ k[:, 0, :, None])[:, :, 0] * SCALE
                + maskp[:, 0:1])                         # (PAIRS, T)
    num = full[:, :, :D] + e0[:, :, None] * v[:, 0][:, None, :]
    den = full[:, :, D] + e0
    out = num / den[:, :, None]

    # global query row: token 0 attends to all keys
    sg = np.einsum('pd,ptd->pt', q[:, 0], k) * SCALE + maskp
    sg -= sg.max(axis=1, keepdims=True)
    eg = np.exp(sg)
    out[:, 0, :] = np.einsum('pt,ptd->pd', eg, v) / eg.sum(axis=1)[:, None]
    return out.reshape(N, H, T, D).astype(np.float32)


def _run(inputs, trace=False, tmpdir=None):
    from concourse.bass_utils import run_bass_kernel_spmd

    q = np.asarray(inputs["query_layer"], np.float32).reshape(PAIRS, T, D)
    k = np.asarray(inputs["key_layer"], np.float32).reshape(PAIRS, T, D)
    v = np.asarray(inputs["value_layer"], np.float32).reshape(PAIRS, T, D)
    mask = np.asarray(inputs["attention_mask"], np.float32).reshape(N, T)

    nc = _build_program()
    in_maps = _prep_core_inputs(q, k, v, mask)
    res = run_bass_kernel_spmd(nc, in_maps, list(range(NCORES)),
                               trace=trace, tmpdir=tmpdir)
    return _postprocess(res.results, q, k, v, mask), res


def kernel(query_layer, key_layer, value_layer, attention_mask):
    out, _ = _run({
        "query_layer": query_layer,
        "key_layer": key_layer,
        "value_layer": value_layer,
        "attention_mask": attention_mask,
    })
    return out


# revision 27
# speedup vs baseline: 1.0021x; 1.0021x over previous
"""Block-local self-attention (BLOCK=128, 3-block windows + global token) on 8
Trainium2 NeuronCores.

Sharding: batch*heads = 32 (n,h) pairs -> 4 pairs per core, no cross-core comms.

Device computes ONLY the block-local window attention, unnormalized:
  - QK: per k-block j (32 slabs), one matmul scoresT[k in j, q in 3 blocks]
    (N=384) from a COMPACT Q^T tile (no host-side 3x replication), with the
    additive mask folded in as a 65th contraction row (K-side row = mask,
    Q-side row = 1.0) and 1/sqrt(d) folded into Q on the host.  The
    contraction dim is zero-padded 65->128 so the full-column weights take
    the PE fast-weight-load path.  Token 0's key is masked out (NEG) so the
    window path excludes it.
  - exp on ScalarE -- the bottleneck engine -- batched 3 slabs/op
    (PSUM->SBUF bf16); QK runs two batches ahead and is hoisted across the
    pair boundary so the exp stream never stalls.
  - PV: ctx[q, 0:64] = sum e*V and ctx[q, 64] = sum e (ones column of V')
    accumulated in PSUM over the 2-3 contributing slabs, 4 windows per PSUM
    group tile; even/odd windows go to different groups so interleaved
    accumulation chains never share a PSUM bank.
  - each finished group is copied PSUM->SBUF by the (otherwise idle) DVE
    into a per-pair out tile and shipped immediately (1KB-row DMAs).

Everything global/tiny runs on the host in numpy instead of burning PE
weight-loads on rank-1 matmuls: the token-0 global-slot term
(out = (ctx + e0*V0) / (den + e0)), the softmax normalization, and the
global-query row (token 0 attends to all keys).
"""

import numpy as np
import ml_dtypes

N, H, T, D = 2, 16, 4000, 64
BLOCK = 128
TP = 4096            # padded token count (32 blocks)
W = 32               # number of 128-blocks
NCORES = 8
PAIRS = N * H        # 32
PPC = PAIRS // NCORES  # pairs per core
NEG = -30000.0
SCALE = 1.0 / np.sqrt(np.float32(D))

# window w -> (group, slot): group = (w%2)*4 + w//8, slot = (w//2)%4
_GRP = [(w % 2) * 4 + w // 8 for w in range(W)]
_SLOT = [(w // 2) % 4 for w in range(W)]
# group -> column position in the out tile, ordered by completion time
# (g0 done at m=3, g4 at m=4, g1 at m=7, ...) so finished halves of the out
# tile can be DMA'd in 2-group chunks while the pair is still computing.
_GORDER = [0, 4, 1, 5, 2, 6, 3, 7]
_GOFF = [_GORDER.index(g) * 260 for g in range(8)]

_prog_cache = {}


def _qlo(j):
    return min(max(j - 1, 0), W - 3)


def _slabs(w):
    return [s for s in (w - 1, w, w + 1) if 0 <= s < W]


def _build_program():
    if "nc" in _prog_cache:
        return _prog_cache["nc"]

    import concourse.bacc as bacc
    import concourse.mybir as mybir
    from concourse import tile

    dt = mybir.dt
    EXP = mybir.ActivationFunctionType.Exp

    nc = bacc.Bacc("TRN2", target_bir_lowering=False, debug=False,
                   num_devices=NCORES)
    # contraction dim padded 65 -> 128 (zero rows): full-column weights
    # enable the PE fast-weight-load path, hiding QK LDWEIGHTS.
    qte_d = nc.dram_tensor("qte", [PPC, 128, TP], dt.bfloat16,
                           kind="ExternalInput").ap()
    kte_d = nc.dram_tensor("kte", [PPC, 128, TP], dt.bfloat16,
                           kind="ExternalInput").ap()
    vp_d = nc.dram_tensor("vp", [PPC, 128, W * 65], dt.bfloat16,
                          kind="ExternalInput").ap()
    out_d = nc.dram_tensor("out", [PPC, 128, 8 * 260], dt.float32,
                           kind="ExternalOutput").ap()

    with tile.TileContext(nc) as tc:
        with (
            tc.tile_pool(name="qte", bufs=3) as qte_pool,
            tc.tile_pool(name="kte", bufs=3) as kte_pool,
            tc.tile_pool(name="vp", bufs=3) as vp_pool,
            tc.tile_pool(name="ex", bufs=4) as ex_pool,
            tc.tile_pool(name="small", bufs=2) as small_pool,
            tc.tile_pool(name="outp", bufs=2) as out_pool,
            tc.tile_pool(name="sc", bufs=2, space="PSUM") as sc_pool,
            tc.tile_pool(name="ctx", bufs=2, space="PSUM") as ctx_pool,
        ):
            def load_pair(p, split=False):
                kte_t = kte_pool.tile([128, TP], dt.bfloat16, tag="kte",
                                      name=f"kte_{p}")
                qte_t = qte_pool.tile([128, TP], dt.bfloat16, tag="qte",
                                      name=f"qte_{p}")
                vp_t = vp_pool.tile([128, W * 65], dt.bfloat16, tag="vp",
                                    name=f"vp_{p}")
                if split:
                    # pair 0 gates the whole pipeline: land the first-QK
                    # columns first so compute starts earlier, and keep the
                    # chunks fine so the QK stream never outruns the DMA
                    # (a >1.7us PE stall here re-throttles the PE clock)
                    for c in range(4):
                        lo, hi = c * 1024, (c + 1) * 1024
                        nc.sync.dma_start(kte_t[:, lo:hi], kte_d[p][:, lo:hi])
                        nc.sync.dma_start(qte_t[:, lo:hi], qte_d[p][:, lo:hi])
                else:
                    nc.sync.dma_start(kte_t[:], kte_d[p])
                    nc.sync.dma_start(qte_t[:], qte_d[p])
                nc.sync.dma_start(vp_t[:], vp_d[p])
                return qte_t, kte_t, vp_t

            # PE warm-up: dense N=512 matmuls on memset data trip the HAM
            # un-throttle (~3.4us of sustained activity) while the first
            # pair's inputs stream in.
            warm_sb = small_pool.tile([128, 512], dt.bfloat16, tag="warm")
            nc.gpsimd.memset(warm_sb[:], 0.25)
            warm_ps = sc_pool.tile([128, 1536], dt.float32, tag="sc",
                                   name="warm_ps")
            for r in range(6):
                nc.tensor.matmul(warm_ps[:, 0:512], warm_sb[:, 0:128],
                                 warm_sb[:, 0:512], start=True, stop=True)

            pending = {0: load_pair(0, split=True)}
            hoisted = {}
            for p in range(PPC):
                qte_t, kte_t, vp_t = pending.pop(p)
                if p + 1 < PPC:
                    pending[p + 1] = load_pair(p + 1)

                out_t = out_pool.tile([128, 8 * 260], dt.float32, tag="out",
                                      name=f"out_{p}")
                ex_tiles = {}
                ctx_tiles = {}

                def emit_qk(b, qte_t=qte_t, kte_t=kte_t, p=p):
                    # scores for slab batch b: k-blocks 3b..min(3b+2, 31)
                    sc = sc_pool.tile([128, 1536], dt.float32, tag="sc",
                                      name=f"sc_{p}_{b}")
                    for h in range(3 if 3 * b + 2 < W else W - 3 * b):
                        j = 3 * b + h
                        lo = _qlo(j) * 128
                        nc.tensor.matmul(
                            sc[:, h * 512:h * 512 + 384],
                            kte_t[:, j * 128:(j + 1) * 128],
                            qte_t[:, lo:lo + 384],
                            start=True, stop=True)
                    return sc

                def emit_exp(b, sc, ex_tiles=ex_tiles, p=p):
                    nb = 3 if 3 * b + 2 < W else W - 3 * b
                    ex = ex_pool.tile([128, 3 * 384], dt.bfloat16, tag="ex",
                                      name=f"ex_{p}_{b}")
                    nc.scalar.activation(
                        ex[:, 0:nb * 384].rearrange("p (b x) -> p b x", x=384),
                        sc[:, 0:nb * 512].rearrange(
                            "p (b x) -> p b x", x=512)[:, :, 0:384],
                        EXP)
                    ex_tiles[b] = ex

                def consume(b, p=p):
                    # windows whose last slab (w+1) landed in batch b
                    ws = [w for w in (3 * b - 1, 3 * b, 3 * b + 1)
                          if 0 <= w < W]
                    # the 1st and 3rd window may share a PSUM group tile:
                    # their accumulation chains must not interleave (two
                    # concurrently open accumulation groups on one bank
                    # corrupt PSUM), so emit the 3rd in its own phase
                    emit_chains(ws[:2])
                    if len(ws) > 2:
                        emit_chains(ws[2:])

                def emit_chains(ws, p=p, vp_t=vp_t, ex_tiles=ex_tiles,
                                ctx_tiles=ctx_tiles, out_t=out_t):
                    seqs = {}
                    for w in ws:
                        g = _GRP[w]
                        if _SLOT[w] == 0:
                            ctx_tiles[g] = ctx_pool.tile(
                                [128, 4 * 65], dt.float32, tag="ctx",
                                name=f"ctx_{p}_{g}")
                        slabs = _slabs(w)
                        seq = []
                        for idx, s in enumerate(slabs):
                            gcol = w - _qlo(s)
                            exm = ex_tiles[s // 3]
                            base = (s % 3) * 384 + gcol * 128
                            seq.append((exm[:, base:base + 128],
                                        vp_t[:, s * 65:(s + 1) * 65],
                                        idx == 0, idx == len(slabs) - 1))
                        seqs[w] = seq
                    # interleave the windows' accumulation chains so
                    # consecutive PE matmuls hit different PSUM banks
                    for r in range(max(len(s) for s in seqs.values())):
                        for w in ws:
                            if r < len(seqs[w]):
                                lhsT, rhs, st, sp = seqs[w][r]
                                g, sl = _GRP[w], _SLOT[w]
                                nc.tensor.matmul(
                                    ctx_tiles[g][:, sl * 65:(sl + 1) * 65],
                                    lhsT, rhs, start=st, stop=sp)
                    for w in ws:
                        g = _GRP[w]
                        if _SLOT[w] == 3:
                            off = _GOFF[g]
                            nc.vector.tensor_copy(
                                out_t[:, off:off + 260], ctx_tiles[g][:])
                            # ship each finished group while the pair is
                            # still computing; rows are 1040B descriptors
                            nc.sync.dma_start(
                                out_d[p][:, off:off + 260],
                                out_t[:, off:off + 260])

                # software pipeline: QK two batches (6 slabs) ahead so the
                # (bottleneck) exp engine never starves on scores.
                NB = (W + 2) // 3
                scs = hoisted.pop(p, None)
                if scs is None:
                    scs = {0: emit_qk(0), 1: emit_qk(1)}
                for b in range(NB):
                    emit_exp(b, scs.pop(b))
                    if b + 2 < NB:
                        scs[b + 2] = emit_qk(b + 2)
                    if 1 <= b <= NB - 2:
                        consume(b - 1)
                # tail: hoist the next pair's first QK batches between the
                # last consumes so the exp engine has no pair-boundary gap
                if p + 1 < PPC:
                    nq, nk, _ = pending[p + 1]
                    h = {0: emit_qk(0, qte_t=nq, kte_t=nk, p=p + 1)}
                    consume(NB - 2)
                    h[1] = emit_qk(1, qte_t=nq, kte_t=nk, p=p + 1)
                    consume(NB - 1)
                    hoisted[p + 1] = h
                else:
                    consume(NB - 2)
                    consume(NB - 1)

    nc.compile()
    _prog_cache["nc"] = nc
    return nc


def _prep_core_inputs(q, k, v, mask):
    """q,k,v: (PAIRS, T, D) f32; mask: (N, T) f32.  Returns list of per-core
    input dicts (bf16 device layouts)."""
    bf16 = ml_dtypes.bfloat16
    maskp = np.repeat(mask, H, axis=0)                   # (PAIRS, T)

    qte = np.zeros((PAIRS, 128, TP), np.float32)
    qte[:, :D, :T] = q.transpose(0, 2, 1) * SCALE
    qte[:, D, :] = 1.0

    kte = np.zeros((PAIRS, 128, TP), np.float32)
    kte[:, :D, :T] = k.transpose(0, 2, 1)
    kte[:, D, :T] = maskp
    kte[:, D, 0] = NEG          # token 0 served by the host global-slot path
    kte[:, D, T:] = NEG

    vp3 = np.zeros((PAIRS, TP, 65), np.float32)
    vp3[:, :T, :D] = v
    vp3[:, :, D] = 1.0
    vp = vp3.reshape(PAIRS, W, 128, 65).transpose(0, 2, 1, 3) \
        .reshape(PAIRS, 128, W * 65)

    qte = qte.astype(bf16)
    kte = kte.astype(bf16)
    vp = vp.astype(bf16)
    return [{
        "qte": qte[c * PPC:(c + 1) * PPC],
        "kte": kte[c * PPC:(c + 1) * PPC],
        "vp": vp[c * PPC:(c + 1) * PPC],
    } for c in range(NCORES)]


def _postprocess(results, q, k, v, mask):
    """Merge the host-side global paths and normalize."""
    maskp = np.repeat(mask, H, axis=0)                   # (PAIRS, T)

    # device windows: (PAIRS, TP, 65) = [sum e*V | sum e]
    o = np.concatenate([results[c]["out"] for c in range(NCORES)], axis=0)
    o = o.reshape(PAIRS, 128, 8, 4, 65)
    pos = [_GOFF[g] // 260 for g in _GRP]
    full = o[:, :, pos, _SLOT, :]                        # (PAIRS, 128, W, 65)
    full = full.transpose(0, 2, 1, 3).reshape(PAIRS, TP, 65)[:, :T]

    # token-0 global slot: every query also attends to k0/v0
    e0 = np.exp((q @ k[:, 0, :, None])[:, :, 0] * SCALE
                + maskp[:, 0:1])                         # (PAIRS, T)
    num = full[:, :, :D] + e0[:, :, None] * v[:, 0][:, None, :]
    den = full[:, :, D] + e0
    out = num / den[:, :, None]

    # global query row: token 0 attends to all keys
    sg = np.einsum('pd,ptd->pt', q[:, 0], k) * SCALE + maskp
    sg -= sg.max(axis=1, keepdims=True)
    eg = np.exp(sg)
    out[:, 0, :] = np.einsum('pt,ptd->pd', eg, v) / eg.sum(axis=1)[:, None]
    return out.reshape(N, H, T, D).astype(np.float32)


def _run(inputs, trace=False, tmpdir=None):
    from concourse.bass_utils import run_bass_kernel_spmd

    q = np.asarray(inputs["query_layer"], np.float32).reshape(PAIRS, T, D)
    k = np.asarray(inputs["key_layer"], np.float32).reshape(PAIRS, T, D)
    v = np.asarray(inputs["value_layer"], np.float32).reshape(PAIRS, T, D)
    mask = np.asarray(inputs["attention_mask"], np.float32).reshape(N, T)

    nc = _build_program()
    in_maps = _prep_core_inputs(q, k, v, mask)
    res = run_bass_kernel_spmd(nc, in_maps, list(range(NCORES)),
                               trace=trace, tmpdir=tmpdir)
    return _postprocess(res.results, q, k, v, mask), res


def kernel(query_layer, key_layer, value_layer, attention_mask):
    out, _ = _run({
        "query_layer": query_layer,
        "key_layer": key_layer,
        "value_layer": value_layer,
        "attention_mask": attention_mask,
    })
    return out
